# revision 10
# baseline (speedup 1.0000x reference)
"""CRF/HMM log-forward (logZ) kernel for Trainium2, 8 NeuronCores.

Math: reference computes, per sentence b,
  la_{j+1}[t] = logsumexp_p(la_j[p] + logA[p,t]) + e_j[t],   j = 0..125
  logZ[b]     = logsumexp_p(la_126[p] + logA[p, eos])
with logA = WA (col bos masked), e_j = WB[:, w_bj], WB = ThetaB @ E.T
(rows bos/eos masked).

Scaled probability space: with Abar = exp(WA - ln 128) (cols bos/eos
zeroed), G_j = exp(WB[:, w_bj]), and M_j = diag(G_j) @ Abar.T,
  logZ = ln( aeos.T @ M_125 @ ... @ M_0 @ a0 ) + 127*ln(128)
where a0 = onehot(bos), aeos = exp(WA[:, eos] - ln 128). The folded 1/128
per-step scale keeps everything in f32 range with no renormalization.

The 126-step product is split into two independent 63-step vector chains
that meet in the middle (halving the latency-bound critical path):
  forward:  a_{j+1} = (Abar.T @ a_j) * G_j          j = 0..62
  backward: v_j     = Abar @ (G_j * v_{j+1})        j = 125..63
  logZ = ln( sum_t a_63[t] * v_63[t] ) + 127*ln(128)
Both chains run in [tag=128 partitions, batch] layout: one 128x128x32
bf16 matmul + one DVE multiply per step, no transposes in the loop.

Sharding: data-parallel over the 256-sentence batch, 32 sentences/core.
Per core: indirect-DMA gather of the 4032 needed E rows with inline
f32->bf16 cast, DMA-transpose (xbar) to [d, occ] layout, 4 K-tile bf16
matmuls + ACT exp to build the emission table G, pipelined against the
scan (G groups are built outside-in so both chains start immediately).
"""

import math

import numpy as np

K = 128
V = 50000
D = 512
BATCH = 256
T_INNER = 126
NCORES = 8
NB = BATCH // NCORES          # 32 sentences per core
NOCC = T_INNER * NB           # 4032 word occurrences per core
NOCC_PAD = 4096               # padded to 32 gather tiles of 128
NTILES = NOCC_PAD // 128      # 32
NGROUPS = 8                   # G built in 8 groups of 512 occurrences
KK = D // 128                 # 4 contraction tiles
BOS = K - 2                   # 126
EOS = K - 1                   # 127
LN_K = math.log(128.0)
FWD_STEPS = T_INNER // 2      # 63
_CACHE = {}


def _build_program():
    import concourse.bacc as bacc
    import concourse.mybir as mybir
    import concourse.tile as tile
    from concourse import bass

    f32 = mybir.dt.float32
    bf16 = mybir.dt.bfloat16
    i32 = mybir.dt.int32
    Exp = mybir.ActivationFunctionType.Exp
    Ln = mybir.ActivationFunctionType.Ln

    nc = bacc.Bacc(
        "TRN2",
        target_bir_lowering=False,
        debug=False,
        num_devices=NCORES,
    )

    e_dram = nc.dram_tensor("e_table", [V, D], f32, kind="ExternalInput").ap()
    tht_dram = nc.dram_tensor("theta_t", [D, K], f32, kind="ExternalInput").ap()
    wa_dram = nc.dram_tensor("wa", [K, K], f32, kind="ExternalInput").ap()
    widx_dram = nc.dram_tensor("widx", [128, NTILES], i32, kind="ExternalInput").ap()
    lz_dram = nc.dram_tensor("logz", [1, NB], f32, kind="ExternalOutput").ap()

    with tile.TileContext(nc) as tc:
        with (
            tc.tile_pool(name="cpool", bufs=1) as cpool,
            tc.tile_pool(name="wpool", bufs=3) as wpool,
            tc.tile_pool(name="gpool", bufs=2) as gpool,
            tc.tile_pool(name="apool", bufs=2) as apool,
            tc.tile_pool(name="ppool", bufs=2, space="PSUM") as ppool,
        ):
            neg_lnk = cpool.tile([128, 1], f32)
            nc.vector.memset(neg_lnk[:], -LN_K)

            wa_sb = cpool.tile([K, K], f32)
            nc.sync.dma_start(out=wa_sb[:], in_=wa_dram[:])
            abar = cpool.tile([K, K], bf16)
            nc.scalar.activation(abar[:], wa_sb[:], Exp, bias=neg_lnk[:])
            # eos column kept aside (unzeroed) for the backward-chain init.
            aeos = cpool.tile([K, 1], bf16)
            nc.scalar.activation(
                aeos[:], wa_sb[:, EOS : EOS + 1], Exp, bias=neg_lnk[:]
            )
            # Zero bos+eos columns: alpha[bos]/alpha[eos] stay 0 at every
            # step, so garbage in G rows bos/eos never propagates.
            nc.vector.memset(abar[:, BOS : BOS + 2], 0.0)
            abar_t = cpool.tile([K, K], bf16)
            nc.sync.dma_start(out=abar_t[:], in_=abar[:], transpose=True)

            # ThetaB.T in bf16, 4 chunks of [128 d, 128 t] side by side.
            tht_f = cpool.tile([128, D], f32)
            for kk in range(KK):
                nc.sync.dma_start(
                    out=tht_f[:, kk * 128 : (kk + 1) * 128],
                    in_=tht_dram[kk * 128 : (kk + 1) * 128, :],
                )
            tht = cpool.tile([128, D], bf16)
            nc.vector.tensor_copy(tht[:], tht_f[:])

            widx = cpool.tile([128, NTILES], i32)
            nc.sync.dma_start(out=widx[:], in_=widx_dram[:])

            ones_bf = cpool.tile([128, 1], bf16)
            nc.vector.memset(ones_bf[:], 1.0)

            g_tab = cpool.tile([K, NOCC_PAD], f32)

            # G is built in 8 groups of 512 occurrence columns. Forward
            # chain consumes groups 0..3, backward chain 7..4; build
            # outside-in pairs interleaved into the scan's program order
            # (priority == program order) so neither chain starves.
            dma_engines = [nc.sync, nc.scalar]
            build_seq = 0

            def build_group(g):
                nonlocal build_seq
                ewt = gpool.tile([128, 4 * D], bf16, tag="ewt", bufs=4)
                for ot_local in range(4):
                    ot = 4 * g + ot_local
                    ew = wpool.tile([128, D], bf16, tag="ew", bufs=8)
                    nc.gpsimd.indirect_dma_start(
                        out=ew[:],
                        out_offset=None,
                        in_=e_dram[:],
                        in_offset=bass.IndirectOffsetOnAxis(
                            ap=widx[:, ot : ot + 1], axis=0
                        ),
                    )
                    # xbar transpose: [128 occ, 512 d] -> [128 d%128,
                    # (kk=d//128, occ)] (probe-verified d = kk*128 + p),
                    # issue alternating across the two HWDGE rings.
                    dma_engines[build_seq % 2].dma_start(
                        out=ewt[:, ot_local * D : (ot_local + 1) * D].rearrange(
                            "p (kk occ) -> p kk occ", kk=KK
                        ),
                        in_=ew[:],
                        transpose=True,
                    )
                    build_seq += 1
                gps = ppool.tile([128, 512], f32, tag="gps")
                ewt4 = ewt[:].rearrange(
                    "p (ot kk occ) -> p ot kk occ", ot=4, kk=KK
                )
                for kk in range(KK):
                    nc.tensor.matmul(
                        gps[:],
                        lhsT=tht[:, kk * 128 : (kk + 1) * 128],
                        rhs=ewt4[:, :, kk, :],
                        start=(kk == 0),
                        stop=(kk == KK - 1),
                    )
                nc.scalar.activation(
                    g_tab[:, g * 512 : (g + 1) * 512], gps[:], Exp
                )

            # First two pairs up front (fwd groups 0,1 / bwd groups 7,6).
            for g in (0, 7, 1, 6):
                build_group(g)

            # ---- forward chain: a_{j+1} = (Abar.T @ a_j) * G_j, j=0..62
            a_t = apool.tile([K, NB], bf16, tag="alpha")
            nc.vector.memset(a_t[:], 0.0)
            nc.gpsimd.affine_select(
                out=a_t[:],
                in_=a_t[:],
                compare_op=mybir.AluOpType.not_equal,
                fill=1.0,
                base=-BOS,
                pattern=[[0, NB]],
                channel_multiplier=1,
            )
            # ---- backward chain: v_j = Abar @ (G_j * v_{j+1}), j=125..63
            v_t = apool.tile([K, NB], bf16, tag="vvec")
            nc.vector.tensor_copy(v_t[:], aeos[:, 0:1].to_broadcast([K, NB]))

            v_cur = v_t[:]  # SBUF on step 0, then PSUM results directly
            for j in range(FWD_STEPS):
                jb = T_INNER - 1 - j  # backward step index, 125..63
                # Prefetch the next group pair one block (16 steps) ahead.
                if j == 8:
                    build_group(2)
                    build_group(5)
                elif j == 24:
                    build_group(3)
                    build_group(4)
                # forward
                fps = ppool.tile([K, NB], f32, tag="fps")
                nc.tensor.matmul(
                    fps[:], lhsT=abar[:], rhs=a_t[:], start=True, stop=True
                )
                a_new = apool.tile([K, NB], bf16, tag="alpha")
                nc.vector.tensor_mul(
                    a_new[:], fps[:], g_tab[:, j * NB : (j + 1) * NB]
                )
                a_t = a_new
                # backward
                u_t = apool.tile([K, NB], bf16, tag="uvec")
                nc.vector.tensor_mul(
                    u_t[:], v_cur, g_tab[:, jb * NB : (jb + 1) * NB]
                )
                vps = ppool.tile([K, NB], f32, tag="vps")
                nc.tensor.matmul(
                    vps[:], lhsT=abar_t[:], rhs=u_t[:], start=True, stop=True
                )
                v_cur = vps[:]

            # meet in the middle: S = sum_t a_63 * v_63
            w_t = apool.tile([K, NB], bf16, tag="wvec")
            nc.vector.tensor_mul(w_t[:], a_t[:], v_cur)
            zps = ppool.tile([1, NB], f32, tag="zps")
            nc.tensor.matmul(
                zps[:], lhsT=ones_bf[:], rhs=w_t[:], start=True, stop=True
            )
            lz = cpool.tile([1, NB], f32)
            nc.scalar.activation(lz[:], zps[:], Ln)
            nc.vector.tensor_scalar_add(lz[:], lz[:], (T_INNER + 1) * LN_K)
            nc.sync.dma_start(out=lz_dram[:], in_=lz[:])

    nc.compile()
    return nc


def _get_program():
    if "nc" not in _CACHE:
        _CACHE["nc"] = _build_program()
    return _CACHE["nc"]


def _make_in_maps(ThetaB, WA, E, words):
    tht = np.ascontiguousarray(ThetaB.T).astype(np.float32, copy=False)
    wa = np.ascontiguousarray(WA).astype(np.float32, copy=False)
    e = np.ascontiguousarray(E).astype(np.float32, copy=False)
    in_maps = []
    for c in range(NCORES):
        ws = words[c * NB : (c + 1) * NB, :]  # [NB, T_INNER]
        occ = np.zeros(NOCC_PAD, dtype=np.int32)
        occ[:NOCC] = ws.T.reshape(-1)  # j-major occurrence order
        widx = np.ascontiguousarray(occ.reshape(NTILES, 128).T)  # [128, NTILES]
        in_maps.append(
            {"e_table": e, "theta_t": tht, "wa": wa, "widx": widx}
        )
    return in_maps


def kernel(ThetaB, WA, E, words, **_unused):
    from concourse.bass_utils import run_bass_kernel_spmd

    nc = _get_program()
    in_maps = _make_in_maps(ThetaB, WA, E, words)
    res = run_bass_kernel_spmd(nc, in_maps, core_ids=list(range(NCORES)))
    out = np.concatenate(
        [res.results[c]["logz"].reshape(-1) for c in range(NCORES)]
    ).astype(np.float32)
    return out


# revision 15
# speedup vs baseline: 2.5613x; 2.5613x over previous
"""CRF/HMM log-forward (logZ) kernel for Trainium2, 8 NeuronCores.

Math: reference computes, per sentence b,
  la_{j+1}[t] = logsumexp_p(la_j[p] + logA[p,t]) + e_j[t],   j = 0..125
  logZ[b]     = logsumexp_p(la_126[p] + logA[p, eos])
with logA = WA (col bos masked), e_j = WB[:, w_bj], WB = ThetaB @ E.T
(rows bos/eos masked).

Scaled probability space: with Abar = exp(WA - ln 128) (cols bos/eos
zeroed), G_j = exp(WB[:, w_bj]), and M_j = diag(G_j) @ Abar.T,
  logZ = ln( aeos.T @ M_125 @ ... @ M_0 @ a0 ) + 127*ln(128)
where a0 = onehot(bos), aeos = exp(WA[:, eos] - ln 128). The folded 1/128
per-step scale keeps everything in f32 range with no renormalization.

The 126-step product is split into two independent 63-step vector chains
that meet in the middle (halving the latency-bound critical path):
  forward:  a_{j+1} = (Abar.T @ a_j) * G_j          j = 0..62
  backward: v_j     = Abar @ (G_j * v_{j+1})        j = 125..63
  logZ = ln( sum_t a_63[t] * v_63[t] ) + 127*ln(128)
Both chains run in [tag=128 partitions, batch] layout: one 128x128x32
bf16 matmul + one DVE multiply per step, no transposes in the loop.

Sharding: data-parallel over the 256-sentence batch, 32 sentences/core.
Per core: indirect-DMA gather of the 4032 needed E rows with inline
f32->bf16 cast (issued up front, in the order the scan consumes them),
PE transposes to [d, occ] layout (bf16, no xbar — DMA-transpose mode
switches serialize against the gather DMAs), 4 K-tile bf16 matmuls +
ACT exp per 512-column group of the emission table G, interleaved with
the scan so both chains are fed just in time.
"""

import math

import numpy as np

K = 128
V = 50000
D = 512
BATCH = 256
T_INNER = 126
NCORES = 8
NB = BATCH // NCORES          # 32 sentences per core
NOCC = T_INNER * NB           # 4032 word occurrences per core
NOCC_PAD = 4096               # padded to 32 gather tiles of 128
NTILES = NOCC_PAD // 128      # 32
KK = D // 128                 # 4 contraction tiles
BOS = K - 2                   # 126
EOS = K - 1                   # 127
LN_K = math.log(128.0)
FWD_STEPS = T_INNER // 2      # 63
_CACHE = {}

# fwd chain consumes groups 0..3, bwd chain 7..4: build outside-in pairs.
PAIR_GROUPS = [(0, 7), (1, 6), (2, 5), (3, 4)]
GATHER_TILE_ORDER = [4 * g + i for pair in PAIR_GROUPS for g in pair for i in range(4)]


def _build_program():
    import concourse.bacc as bacc
    import concourse.mybir as mybir
    import concourse.tile as tile
    from concourse import bass
    from concourse.masks import make_identity

    f32 = mybir.dt.float32
    bf16 = mybir.dt.bfloat16
    i32 = mybir.dt.int32
    Exp = mybir.ActivationFunctionType.Exp
    Ln = mybir.ActivationFunctionType.Ln

    nc = bacc.Bacc(
        "TRN2",
        target_bir_lowering=False,
        debug=False,
        num_devices=NCORES,
    )

    e_dram = nc.dram_tensor("e_table", [V, D], f32, kind="ExternalInput").ap()
    tht_dram = nc.dram_tensor("theta_t", [D, K], f32, kind="ExternalInput").ap()
    wa_dram = nc.dram_tensor("wa", [K, K], f32, kind="ExternalInput").ap()
    widx_dram = nc.dram_tensor("widx", [128, NTILES], i32, kind="ExternalInput").ap()
    lz_dram = nc.dram_tensor("logz", [1, NB], f32, kind="ExternalOutput").ap()

    with tile.TileContext(nc) as tc:
        with (
            tc.tile_pool(name="cpool", bufs=1) as cpool,
            tc.tile_pool(name="wpool", bufs=3) as wpool,
            tc.tile_pool(name="gpool", bufs=3) as gpool,
            tc.tile_pool(name="apool", bufs=2) as apool,
            tc.tile_pool(name="ppool", bufs=2, space="PSUM") as ppool,
        ):
            # --- gathers first: the serial SWDGE descriptor generation is
            # the long pole, so the pool queue must start on it at once.
            widx = cpool.tile([128, NTILES], i32)
            nc.sync.dma_start(out=widx[:], in_=widx_dram[:])
            ew_tiles = {}
            for ot in GATHER_TILE_ORDER:
                ew = wpool.tile([128, D], bf16, tag="ew", bufs=16, name=f"ew{ot}")
                nc.gpsimd.indirect_dma_start(
                    out=ew[:],
                    out_offset=None,
                    in_=e_dram[:],
                    in_offset=bass.IndirectOffsetOnAxis(
                        ap=widx[:, ot : ot + 1], axis=0
                    ),
                )
                ew_tiles[ot] = ew

            identity = cpool.tile([128, 128], bf16)
            make_identity(nc, identity)

            neg_lnk = cpool.tile([128, 1], f32)
            nc.vector.memset(neg_lnk[:], -LN_K)

            wa_sb = cpool.tile([K, K], f32)
            nc.sync.dma_start(out=wa_sb[:], in_=wa_dram[:])
            abar_f = cpool.tile([K, K], f32)
            nc.scalar.activation(abar_f[:], wa_sb[:], Exp, bias=neg_lnk[:])
            # eos column kept aside (unzeroed) for the backward-chain init.
            aeos = cpool.tile([K, 1], bf16)
            nc.scalar.activation(
                aeos[:], wa_sb[:, EOS : EOS + 1], Exp, bias=neg_lnk[:]
            )
            # Zero bos+eos columns: alpha[bos]/alpha[eos] stay 0 at every
            # step, so garbage in G rows bos/eos never propagates.
            nc.vector.memset(abar_f[:, BOS : BOS + 2], 0.0)
            abar = cpool.tile([K, K], bf16)
            nc.vector.tensor_copy(abar[:], abar_f[:])
            atp = ppool.tile([K, K], f32, tag="gps", bufs=1)
            idf = cpool.tile([128, 128], f32)
            make_identity(nc, idf)
            nc.tensor.transpose(out=atp[:], in_=abar_f[:], identity=idf[:])
            abar_t = cpool.tile([K, K], bf16)
            nc.vector.tensor_copy(abar_t[:], atp[:])

            # ThetaB.T in bf16, 4 chunks of [128 d, 128 t] side by side.
            tht_f = cpool.tile([128, D], f32)
            for kk in range(KK):
                nc.sync.dma_start(
                    out=tht_f[:, kk * 128 : (kk + 1) * 128],
                    in_=tht_dram[kk * 128 : (kk + 1) * 128, :],
                )
            tht = cpool.tile([128, D], bf16)
            nc.vector.tensor_copy(tht[:], tht_f[:])

            ones_bf = cpool.tile([128, 1], bf16)
            nc.vector.memset(ones_bf[:], 1.0)

            g_tab = cpool.tile([K, NOCC_PAD], f32)
            build_seq = 0

            def build_group(g):
                nonlocal build_seq
                ewt = gpool.tile([128, 4 * D], bf16, tag="ewt")
                for ot_local in range(4):
                    ot = 4 * g + ot_local
                    ew = ew_tiles[ot]
                    trp = ppool.tile([128, D], bf16, tag="trp")
                    for kk in range(KK):
                        nc.tensor.transpose(
                            out=trp[:, kk * 128 : (kk + 1) * 128],
                            in_=ew[:, kk * 128 : (kk + 1) * 128],
                            identity=identity[:],
                        )
                    # alternate the PSUM->SBUF copies across DVE and ACT
                    if build_seq % 2 == 0:
                        nc.vector.tensor_copy(
                            ewt[:, ot_local * D : (ot_local + 1) * D], trp[:]
                        )
                    else:
                        nc.scalar.copy(
                            ewt[:, ot_local * D : (ot_local + 1) * D], trp[:]
                        )
                    build_seq += 1
                gps = ppool.tile([128, 512], f32, tag="gps", bufs=1)
                ewt4 = ewt[:].rearrange(
                    "p (ot kk occ) -> p ot kk occ", ot=4, kk=KK
                )
                for kk in range(KK):
                    nc.tensor.matmul(
                        gps[:],
                        lhsT=tht[:, kk * 128 : (kk + 1) * 128],
                        rhs=ewt4[:, :, kk, :],
                        start=(kk == 0),
                        stop=(kk == KK - 1),
                    )
                nc.scalar.activation(
                    g_tab[:, g * 512 : (g + 1) * 512], gps[:], Exp
                )

            for g in PAIR_GROUPS[0] + PAIR_GROUPS[1]:
                build_group(g)

            # ---- forward chain init: a0 = onehot(bos)
            a_t = apool.tile([K, NB], bf16, tag="alpha")
            nc.vector.memset(a_t[:], 0.0)
            nc.gpsimd.affine_select(
                out=a_t[:],
                in_=a_t[:],
                compare_op=mybir.AluOpType.not_equal,
                fill=1.0,
                base=-BOS,
                pattern=[[0, NB]],
                channel_multiplier=1,
            )
            # ---- backward chain init: v_126 = aeos broadcast
            v_t = apool.tile([K, NB], bf16, tag="vvec")
            nc.vector.tensor_copy(v_t[:], aeos[:, 0:1].to_broadcast([K, NB]))

            v_cur = v_t[:]  # SBUF on step 0, then PSUM results directly
            for j in range(FWD_STEPS):
                jb = T_INNER - 1 - j  # backward step index, 125..63
                # Build the next group pair one block (16 steps) ahead.
                if j == 8:
                    for g in PAIR_GROUPS[2]:
                        build_group(g)
                elif j == 24:
                    for g in PAIR_GROUPS[3]:
                        build_group(g)
                # forward
                fps = ppool.tile([K, NB], f32, tag="fps")
                nc.tensor.matmul(
                    fps[:], lhsT=abar[:], rhs=a_t[:], start=True, stop=True
                )
                a_new = apool.tile([K, NB], bf16, tag="alpha")
                nc.vector.tensor_mul(
                    a_new[:], fps[:], g_tab[:, j * NB : (j + 1) * NB]
                )
                a_t = a_new
                # backward
                u_t = apool.tile([K, NB], bf16, tag="uvec")
                nc.vector.tensor_mul(
                    u_t[:], v_cur, g_tab[:, jb * NB : (jb + 1) * NB]
                )
                vps = ppool.tile([K, NB], f32, tag="vps")
                nc.tensor.matmul(
                    vps[:], lhsT=abar_t[:], rhs=u_t[:], start=True, stop=True
                )
                v_cur = vps[:]

            # meet in the middle: S = sum_t a_63 * v_63
            w_t = apool.tile([K, NB], bf16, tag="wvec")
            nc.vector.tensor_mul(w_t[:], a_t[:], v_cur)
            zps = ppool.tile([1, NB], f32, tag="fps")
            nc.tensor.matmul(
                zps[:], lhsT=ones_bf[:], rhs=w_t[:], start=True, stop=True
            )
            lz = cpool.tile([1, NB], f32)
            nc.scalar.activation(lz[:], zps[:], Ln)
            nc.vector.tensor_scalar_add(lz[:], lz[:], (T_INNER + 1) * LN_K)
            nc.sync.dma_start(out=lz_dram[:], in_=lz[:])

    nc.compile()
    return nc


def _get_program():
    if "nc" not in _CACHE:
        _CACHE["nc"] = _build_program()
    return _CACHE["nc"]


def _make_in_maps(ThetaB, WA, E, words):
    tht = np.ascontiguousarray(ThetaB.T).astype(np.float32, copy=False)
    wa = np.ascontiguousarray(WA).astype(np.float32, copy=False)
    e = np.ascontiguousarray(E).astype(np.float32, copy=False)
    in_maps = []
    for c in range(NCORES):
        ws = words[c * NB : (c + 1) * NB, :]  # [NB, T_INNER]
        occ = np.zeros(NOCC_PAD, dtype=np.int32)
        occ[:NOCC] = ws.T.reshape(-1)  # j-major occurrence order
        widx = np.ascontiguousarray(occ.reshape(NTILES, 128).T)  # [128, NTILES]
        in_maps.append(
            {"e_table": e, "theta_t": tht, "wa": wa, "widx": widx}
        )
    return in_maps


def kernel(ThetaB, WA, E, words, **_unused):
    from concourse.bass_utils import run_bass_kernel_spmd

    nc = _get_program()
    in_maps = _make_in_maps(ThetaB, WA, E, words)
    res = run_bass_kernel_spmd(nc, in_maps, core_ids=list(range(NCORES)))
    out = np.concatenate(
        [res.results[c]["logz"].reshape(-1) for c in range(NCORES)]
    ).astype(np.float32)
    return out
